# revision 1
# baseline (speedup 1.0000x reference)
"""Trainium kernel for nn_AttnModel_75118978007374 (temporal graph attention).

Strategy: pure data parallel over B across the 8 NeuronCores (axon PJRT
devices). The math is algebraically restructured so the per-row attention
never materializes the full K/V projections:

    scores[b,h,n] = (k_in[b,n,:] @ Wk[:,hb]) . (q_in[b] @ Wq)[hb] / sqrt(DK)
                  = k_in[b,n,:] . p[b,h,:],   p = q_in @ (Wq[:,hb] @ Wk[:,hb].T)
    ctx[b,h,:]    = (attn[b,h,:] @ k_in[b])   @ Wv[:,hb]

This turns the [B*N,384]x[384,384] K/V projections (~310 GFLOP) into
[B,384]x[384,384]-sized ops (~12 GFLOP total), leaving the kernel
memory-bound (reads 776 MB), which matches the target regime.
"""

import numpy as np

B, N, D = 8192, 64, 128
H = 2
DM = 3 * D
DK = DM // H
LN_EPS = 1e-5
NEG = -1e10
NCORES = 8
BS = B // NCORES  # 1024 rows per core

_compiled = None


def _build():
    import jax
    import jax.numpy as jnp

    def shard_fn(src, src_t, seq, seq_t, seq_e, mask,
                 Wq, Wk, Wv, Wfc, bfc, ln_g, ln_b, W1, b1, W2, b2):
        q_in = jnp.concatenate(
            [src, jnp.zeros_like(src), src_t[:, 0, :]], axis=1)      # [Bs,DM]
        k_in = jnp.concatenate([seq, seq_e, seq_t], axis=2)          # [Bs,N,DM]

        scale = 1.0 / np.sqrt(DK)
        ps, ctxs, attns = [], [], []
        for h in range(H):
            hb = slice(h * DK, (h + 1) * DK)
            A_h = (Wq[:, hb] @ Wk[:, hb].T) * scale                  # [DM,DM]
            ps.append(q_in @ A_h)                                    # [Bs,DM]
        for h in range(H):
            s = jnp.einsum("bd,bnd->bn", ps[h], k_in)                # [Bs,N]
            s = jnp.where(mask, NEG, s)
            s = s - s.max(axis=1, keepdims=True)
            e = jnp.exp(s)
            a = e / e.sum(axis=1, keepdims=True)                     # [Bs,N]
            attns.append(a)
            w = jnp.einsum("bn,bnd->bd", a, k_in)                    # [Bs,DM]
            ctxs.append(w @ Wv[:, h * DK:(h + 1) * DK])              # [Bs,DK]
        ctx = jnp.concatenate(ctxs, axis=1)                          # [Bs,DM]

        out = ctx @ Wfc + bfc + q_in
        mu = out.mean(axis=-1, keepdims=True)
        var = ((out - mu) ** 2).mean(axis=-1, keepdims=True)
        out = (out - mu) / jnp.sqrt(var + LN_EPS) * ln_g + ln_b

        attn_out = jnp.stack(attns, axis=0).reshape(H * src.shape[0], N)

        hmlp = jax.nn.relu(jnp.concatenate([out, src], axis=1) @ W1 + b1)
        merged = hmlp @ W2 + b2
        return merged, attn_out

    return shard_fn


def _get_compiled():
    global _compiled
    if _compiled is None:
        import jax
        shard_fn = _build()
        devs = jax.devices()[:NCORES]
        _compiled = (jax, [jax.jit(shard_fn, device=d) for d in devs], devs)
    return _compiled


def kernel(**inputs):
    src = np.asarray(inputs["src"], np.float32)
    src_t = np.asarray(inputs["src_t"], np.float32)
    seq = np.asarray(inputs["seq"], np.float32)
    seq_t = np.asarray(inputs["seq_t"], np.float32)
    seq_e = np.asarray(inputs["seq_e"], np.float32)
    mask = np.asarray(inputs["mask"])
    wnames = ["Wq", "Wk", "Wv", "Wfc", "bfc", "ln_g", "ln_b",
              "W1", "b1", "W2", "b2"]
    ws = [np.asarray(inputs[k], np.float32) for k in wnames]

    jax, fns, devs = _get_compiled()

    outs = []
    for i in range(NCORES):
        sl = slice(i * BS, (i + 1) * BS)
        outs.append(fns[i](src[sl], src_t[sl], seq[sl], seq_t[sl],
                           seq_e[sl], mask[sl], *ws))
    merged = np.concatenate([np.asarray(o[0]) for o in outs], axis=0)
    # per-shard attn_out is [H*BS, N] heads-major; reassemble to [H*B, N]
    attn = np.concatenate(
        [np.asarray(o[1]).reshape(H, BS, N) for o in outs], axis=1
    ).reshape(H * B, N)
    return merged.astype(np.float32), attn.astype(np.float32)
